# revision 40
# baseline (speedup 1.0000x reference)
"""Attention-Augmented Conv2D (AAConv2D) distributed Bass kernel for 8 TRN2 NeuronCores.

Strategy: pure data-parallel over batch (B=8 -> one image per core, weights
replicated, zero collectives). Per core, for one [32,32,256] image:

  conv branch : 3x3 SAME conv (256->256ch) as 9 shifted-window matmuls
                accumulated in PSUM, channel-major output.
  attn branch : kqv 1x1 conv (channel-major k/q, position-major v),
                per-head S^T = K Q^T computed with an AUGMENTED contraction
                (K = 32 qk dims + 32 one-hot w-offset dims + 32 one-hot
                h-offset dims = 96) so the relative-position logits ride in
                the same matmul stream; exp on ScalarE (max-free softmax,
                logits are O(10) so fp32 exp is safe); P^T V via matmul with
                [v|1] stationary (sumexp ride-along); per-head normalize on
                VectorE; output projection.

All heavy matmuls in bf16 (fp32 matmul is 4x slower on TRN2 PE).
Host does layout-only prep: batch sharding, transposes to channel-major,
bf16 casts, relative-table window expansion, one-hot delta tables, and the
exact algebraic fold of the v-bias into the projection bias.
"""

import contextlib

import numpy as np
import ml_dtypes

BF16 = ml_dtypes.bfloat16

B, H, W, FIN = 8, 32, 32, 256
POS = H * W
FOUT, K, DK, DV, NH = 512, 3, 256, 256, 8
DKH, DVH = DK // NH, DV // NH
FOUT_CONV = FOUT - DV  # 256
N_CORES = 8

_PROG_CACHE = {}


def _build_program(variant="full"):
    """Build (and cache) the compiled Bass program. Same program for all 8
    cores (SPMD); per-core data arrives via the per-core input maps.

    variant: bisect knob —
      "noatt"  : skip the whole per-head attention loop
      "norel"  : attention without relative-logit matmuls
      "nonorm" : attention + rel, but plain copy instead of normalize
      "full"   : everything
    """
    if ("nc", variant) in _PROG_CACHE:
        return _PROG_CACHE[("nc", variant)]

    import concourse.bass as bass
    import concourse.bacc as bacc
    import concourse.tile as tile
    from concourse import mybir

    BF = mybir.dt.bfloat16
    F32 = mybir.dt.float32
    EXP = mybir.ActivationFunctionType.Exp

    nc = bacc.Bacc("TRN2", target_bir_lowering=False, debug=False,
                   num_devices=N_CORES)

    # ---- DRAM parameters (per-core shapes; all laid out for contiguous
    # per-partition DMA runs — descriptor count is a real startup cost) ----
    PADW = H + 2  # 34
    xpad_d = nc.dram_tensor("xpad", [FIN, PADW * PADW], BF, kind="ExternalInput")
    xtc_d = nc.dram_tensor("xtc", [FIN, POS], BF, kind="ExternalInput")
    wkqv_d = nc.dram_tensor("wkqv", [FIN, 2 * DK + DV], BF, kind="ExternalInput")
    wconv_d = nc.dram_tensor("wconv", [2, 128, K * K * FOUT_CONV], BF,
                             kind="ExternalInput")
    wproj_d = nc.dram_tensor("wproj", [DV, DV], BF, kind="ExternalInput")
    biases_d = nc.dram_tensor("biases", [128, 8], F32, kind="ExternalInput")
    krw_d = nc.dram_tensor("krw4", [128, 1024], BF, kind="ExternalInput")
    krh_d = nc.dram_tensor("krh4", [128, 1024], BF, kind="ExternalInput")
    delta_d = nc.dram_tensor("delta", [2 * W, POS], BF, kind="ExternalInput")
    out_d = nc.dram_tensor("out", [FOUT, POS], F32, kind="ExternalOutput")

    xpad_in = xpad_d.ap()
    xtc = xtc_d.ap()
    wkqv = wkqv_d.ap()
    wconv = wconv_d.ap()
    wproj = wproj_d.ap()
    biases = biases_d.ap()
    krw = krw_d.ap()
    krh = krh_d.ap()
    delta = delta_d.ap()
    out = out_d.ap()

    with tile.TileContext(nc) as tc, contextlib.ExitStack() as ctx:
        consts = ctx.enter_context(tc.tile_pool(name="consts", bufs=1))
        xpads = ctx.enter_context(tc.tile_pool(name="xpads", bufs=1))
        kqpool = ctx.enter_context(tc.tile_pool(name="kqpool", bufs=1))
        vopool = ctx.enter_context(tc.tile_pool(name="vopool", bufs=1))
        attall = ctx.enter_context(tc.tile_pool(name="attall", bufs=1))
        work = ctx.enter_context(tc.tile_pool(name="work", bufs=4))
        small = ctx.enter_context(tc.tile_pool(name="small", bufs=3))
        outp = ctx.enter_context(tc.tile_pool(name="outp", bufs=2))
        ps_big = ctx.enter_context(tc.tile_pool(name="ps_big", bufs=2,
                                                space=bass.MemorySpace.PSUM))
        ps_rp = ctx.enter_context(tc.tile_pool(name="ps_rp", bufs=1,
                                               space=bass.MemorySpace.PSUM))
        ps_at = ctx.enter_context(tc.tile_pool(name="ps_at", bufs=1,
                                               space=bass.MemorySpace.PSUM))

        # ---- zero-padded input (pre-padded on host), channel-major ----
        xpad = []
        xt_sb = []
        for f in range(2):
            t = xpads.tile([128, PADW * PADW], BF, tag=f"xpad{f}")
            nc.sync.dma_start(out=t[:], in_=xpad_in[f * 128:(f + 1) * 128, :])
            xpad.append(t)
            # unpadded copy: contiguous position axis (for matmul stationaries)
            tu = xpads.tile([128, POS], BF, tag=f"xtsb{f}")
            nc.sync.dma_start(out=tu[:], in_=xtc[f * 128:(f + 1) * 128, :])
            xt_sb.append(tu)

        # ---- constants into SBUF ----
        wkqv_sb = []
        for f in range(2):
            t = consts.tile([128, 2 * DK + DV], BF, tag=f"wkqv{f}")
            nc.sync.dma_start(out=t[:], in_=wkqv[f * 128:(f + 1) * 128, :])
            wkqv_sb.append(t)
        wconv_sb = []
        for f in range(2):
            t = consts.tile([128, K * K * FOUT_CONV], BF, tag=f"wconv{f}")
            nc.sync.dma_start(out=t[:], in_=wconv[f, :, :])
            wconv_sb.append(t)
        wproj_sb = []
        for f in range(2):
            t = consts.tile([128, DV], BF, tag=f"wproj{f}")
            nc.sync.dma_start(out=t[:], in_=wproj[f * 128:(f + 1) * 128, :])
            wproj_sb.append(t)
        krw_sb = consts.tile([128, 1024], BF, tag="krw")
        nc.sync.dma_start(out=krw_sb[:], in_=krw[:, :])
        krh_sb = consts.tile([128, 1024], BF, tag="krh")
        nc.sync.dma_start(out=krh_sb[:], in_=krh[:, :])

        # one combined per-partition bias tile: cols 0-3 = b_kq chunks,
        # 4-5 = b_conv chunks, 6-7 = effective b_proj chunks
        ball_sb = consts.tile([128, 8], F32, tag="ball")
        nc.sync.dma_start(out=ball_sb[:], in_=biases[:, :])
        bkq_sb = [ball_sb[:, cc:cc + 1] for cc in range(4)]
        bconv_sb = [ball_sb[:, 4 + co:5 + co] for co in range(2)]
        bproj_sb = [ball_sb[:, 6 + co:7 + co] for co in range(2)]

        def xwin(f, dy, dx, h0, hn):
            # [128, hn, 32] window of the padded image: rows h0..h0+hn of the
            # conv-tap (dy,dx)-shifted image, all 32 columns.
            t3 = xpad[f].rearrange("p (a b) -> p a b", a=PADW)
            return t3[:, h0 + dy:h0 + dy + hn, dx:dx + W]

        # ---- kqv: k and q sections, channel-major [co, pos] ----
        kq_sb = []
        for cc in range(4):
            ps = ps_big.tile([128, POS], F32, tag="psbig")
            for f in range(2):
                for nh in range(2):
                    nc.tensor.matmul(
                        ps[:, nh * 512:(nh + 1) * 512],
                        lhsT=wkqv_sb[f][:, cc * 128:(cc + 1) * 128],
                        rhs=xwin(f, 1, 1, nh * 16, 16),
                        start=(f == 0), stop=(f == 1))
            t = kqpool.tile([128, POS], BF, tag=f"kq{cc}")
            nc.vector.tensor_scalar_add(out=t[:], in0=ps[:], scalar1=bkq_sb[cc][:])
            kq_sb.append(t)

        # ---- v: position-major [pos, dv], with interleaved ones column ----
        vo_sb = []
        for kc in range(8):
            ps = ps_big.tile([128, DV], F32, tag="psbig")
            for f in range(2):
                nc.tensor.matmul(
                    ps[:],
                    lhsT=xt_sb[f][:, kc * 128:(kc + 1) * 128],
                    rhs=wkqv_sb[f][:, 2 * DK:2 * DK + DV],
                    start=(f == 0), stop=(f == 1))
            vo = vopool.tile([128, NH * (DVH + 1)], BF, tag=f"vo{kc}")
            vo3 = vo.rearrange("p (h d) -> p h d", d=DVH + 1)
            nc.vector.memset(vo3[:, :, DVH:DVH + 1], 1.0)
            nc.vector.tensor_copy(
                out=vo3[:, :, 0:DVH],
                in_=ps.rearrange("p (h d) -> p h d", d=DVH))
            vo_sb.append(vo)

        att_all = []
        for f in range(2):
            t = attall.tile([128, POS], BF, tag=f"att{f}", name=f"att{f}")
            att_all.append(t)

        # three persistent big-matmul stationaries (rows 32-95 = constant
        # one-hot deltas, loaded once; rows 0-31 swapped per head; three so
        # the prep-ahead depth of 2 never overwrites a stationary in use)
        st_trio = []
        for i in range(3):
            t = attall.tile([128, POS], BF, tag=f"stp{i}", name=f"stp{i}")
            nc.sync.dma_start(out=t[32:96, :], in_=delta[:, :])
            st_trio.append(t)
        ones_sb = consts.tile([1, 32], BF, tag="ones")
        nc.vector.memset(ones_sb[:], 1.0)

        # ---- per-head attention (software-pipelined: prep h+1 ahead) ----
        def prep_head(h):
            """rel matmuls + rhs/stationary assembly for head h."""
            sec = h // 4          # 128-channel chunk of the k/q section
            g = (h % 4) * 32      # partition offset of this head inside it

            # qT replicated to partition groups 0/1 (for row-tiled rel mms)
            qrep = work.tile([128, POS], BF, tag="qrep", name=f"qrep{h}")
            for r in range(2):
                nc.sync.dma_start(out=qrep[32 * r:32 * r + 32, :],
                                  in_=kq_sb[2 + sec][g:g + 32, :])

            # relative-position logit tables, gathered per (offset, q):
            #   rp[32+w', ...] = sum_d krw[d, wq*32+w'] qT[d, hq*32+wq]
            #   rp[64+h', ...] = sum_d krh[d, hq*32+h'] qT[d, hq*32+wq]
            # 2-way row-tiled packing; CONSTRAINT: concurrent tiles in the
            # same PE column strip must drain to DIFFERENT psum banks, so
            # the psum column is 512*(row group) + 32*(index//2).
            rp = ps_rp.tile([128, POS], F32, tag="rp", name=f"rp{h}")
            qrep3 = qrep.rearrange("p (a b) -> p b a", b=W)  # [128, wq, hq]
            for wq in range(W):
                r = wq % 2
                col = 512 * r + 32 * (wq // 2)
                nc.tensor.matmul(
                    rp[32:64, col:col + 32],
                    lhsT=krw_sb[32 * r:32 * r + 32, wq * 32:(wq + 1) * 32],
                    rhs=qrep3[32 * r:32 * r + 32, wq, :],
                    start=True, stop=True, tile_position=(32 * r, 32))
            for hq in range(H):
                r = hq % 2
                col = 512 * r + 32 * (hq // 2)
                nc.tensor.matmul(
                    rp[64:96, col:col + 32],
                    lhsT=krh_sb[32 * r:32 * r + 32, hq * 32:(hq + 1) * 32],
                    rhs=qrep[32 * r:32 * r + 32, hq * 32:(hq + 1) * 32],
                    start=True, stop=True, tile_position=(32 * r, 64))

            # rhs rows: 0-31 qT, 32-63 RWg, 64-95 RHg — un-permute into
            # q-major (one DVE copy per bank):
            #  rel_w: src col = 512b + 32a + hq  (wq = 2a + b); dst 32*hq + wq
            #  rel_h: src col = 512b + 32a + wq  (hq = 2a + b); dst 64a+32b+wq
            rh = work.tile([128, POS], BF, tag="rh", name=f"rh{h}")
            nc.sync.dma_start(out=rh[0:32, :], in_=kq_sb[2 + sec][g:g + 32, :])
            rw_src = rp[32:64, :].rearrange("p (b a c) -> p b c a", b=2, a=16)
            rw_dst = rh[32:64, :].rearrange("p (c a b) -> p b c a", a=16, b=2)
            rh_src = rp[64:96, :].rearrange("p (b a c) -> p b a c", b=2, a=16)
            rh_dst = rh[64:96, :].rearrange("p (a b c) -> p b a c", a=16, b=2)
            for bb in range(2):
                nc.vector.tensor_copy(out=rw_dst[:, bb], in_=rw_src[:, bb])
                nc.vector.tensor_copy(out=rh_dst[:, bb], in_=rh_src[:, bb])

            # stationary: swap in this head's kT rows (deltas persist)
            st = st_trio[h % 3]
            nc.sync.dma_start(out=st[0:32, :], in_=kq_sb[sec][g:g + 32, :])
            return rh, st

        def inner_head(h, rh, st):
            sec = h // 4
            g = (h % 4) * 32
            at = ps_at.tile([DVH + 1, POS], F32, tag="at", name=f"at{h}")
            # skewed S/PV emission: PE never sits behind the exp of the
            # chunk it just produced (S(kc+1) runs while ACT does exp(kc))
            sps_t = [None] * 8
            psb_t = [None] * 8

            def s_step(kc):
                sps = ps_big.tile([128, POS], F32, tag="psbig",
                                  name=f"sps{h}_{kc}")
                for nh in range(2):
                    nc.tensor.matmul(
                        sps[:, nh * 512:(nh + 1) * 512],
                        lhsT=st[0:96, kc * 128:(kc + 1) * 128],
                        rhs=rh[0:96, nh * 512:(nh + 1) * 512],
                        start=True, stop=True)
                psb = work.tile([128, POS], BF, tag="pexp", name=f"psb{h}_{kc}")
                nc.scalar.activation(out=psb[:], in_=sps[:], func=EXP)
                psb_t[kc] = psb

            def pv_step(kc):
                for nh in range(2):
                    nc.tensor.matmul(
                        at[:, nh * 512:(nh + 1) * 512],
                        lhsT=vo_sb[kc][:, h * (DVH + 1):(h + 1) * (DVH + 1)],
                        rhs=psb_t[kc][:, nh * 512:(nh + 1) * 512],
                        start=(kc == 0), stop=(kc == 7))

            s_step(0)
            for kc in range(1, 8):
                s_step(kc)
                pv_step(kc - 1)
            pv_step(7)

            # normalize: attn_h = (P^T V)[0:32] / sumexp (row 32).
            # One combined psum-escape copy frees `at` immediately; the
            # reciprocal runs on a [128, 8] reshape (DVE reciprocal cost is
            # free-dim-driven: [1,1024] would cost ~6.5us, [128,8] ~0.25us).
            if variant == "oldnorm":
                an = small.tile([32, POS], BF, tag="an", name=f"an{h}")
                rcp = small.tile([1, POS], F32, tag="rcp", name=f"rcp{h}")
                nc.vector.reciprocal(out=rcp[:], in_=at[DVH:DVH + 1, :])
                rcpb = small.tile([32, POS], F32, tag="rcpb", name=f"rcpb{h}")
                nc.gpsimd.partition_broadcast(rcpb[:], rcp[:])
                nc.vector.tensor_mul(an[:], at[0:DVH, :], rcpb[:])
                nc.sync.dma_start(out=att_all[sec][g:g + 32, :], in_=an[:])
                return
            cmb = small.tile([DVH + 1, POS], BF, tag="cmb", name=f"cmb{h}")
            nc.vector.tensor_copy(out=cmb[:], in_=at[:])
            s8 = small.tile([128, 8], BF, tag="s8", name=f"s8{h}")
            nc.gpsimd.dma_start(out=s8[:], in_=cmb[DVH:DVH + 1, :])
            rcp8 = small.tile([128, 8], BF, tag="rcp8", name=f"rcp8{h}")
            with nc.allow_low_precision(reason="1/sumexp in bf16 is within "
                                        "the softmax rounding budget"):
                nc.vector.reciprocal(out=rcp8[:], in_=s8[:])
            rcpf = small.tile([1, POS], BF, tag="rcpf", name=f"rcpf{h}")
            nc.gpsimd.dma_start(out=rcpf[:], in_=rcp8[:])
            # partition-broadcast 1/sumexp via a K=1 rank-1 matmul (the
            # GpSimd PartitionBroadcast op costs ~1.8us; this is ~0.5us on PE)
            rcpp = ps_rp.tile([32, POS], F32, tag="rp", name=f"rcpp{h}")
            for nh in range(2):
                nc.tensor.matmul(rcpp[:, nh * 512:(nh + 1) * 512],
                                 lhsT=ones_sb[:],
                                 rhs=rcpf[:, nh * 512:(nh + 1) * 512],
                                 start=True, stop=True)
            an = small.tile([32, POS], BF, tag="an", name=f"an{h}")
            nc.vector.tensor_mul(an[:], cmb[0:DVH, :], rcpp[:])
            nc.gpsimd.dma_start(out=att_all[sec][g:g + 32, :], in_=an[:])

        # ---- conv branch (emitted before attention: fills the post-kqv PE
        # gap, keeps HAM warm, and gets its output DMA off the kernel tail) ----
        for co in range(2):
            ps = ps_big.tile([128, POS], F32, tag="psbig", name=f"cps{co}")
            for nh in range(2):
                idx = 0
                for tp in range(9):
                    dy, dx = tp // 3, tp % 3
                    for f in range(2):
                        o0 = tp * FOUT_CONV + co * 128
                        nc.tensor.matmul(
                            ps[:, nh * 512:(nh + 1) * 512],
                            lhsT=wconv_sb[f][:, o0:o0 + 128],
                            rhs=xwin(f, dy, dx, nh * 16, 16),
                            start=(idx == 0), stop=(idx == 17))
                        idx += 1
            ot = outp.tile([128, POS], F32, tag="out", name=f"cot{co}")
            nc.vector.tensor_scalar_add(out=ot[:], in0=ps[:], scalar1=bconv_sb[co][:])
            nc.sync.dma_start(out=out[co * 128:(co + 1) * 128, :], in_=ot[:])

        if variant != "noatt":
            # prep TWO heads ahead so the next head's PE work never waits on
            # the current head's normalize tail
            pq = [prep_head(0), prep_head(1)]
            for h in range(8):
                if h + 2 < 8:
                    pq.append(prep_head(h + 2))
                inner_head(h, *pq[h])
        else:
            for t in att_all:
                nc.vector.memset(t[:], 0.0)

        # ---- output projection ----
        for co in range(2):
            ps = ps_big.tile([128, POS], F32, tag="psbig")
            for f in range(2):
                for nh in range(2):
                    nc.tensor.matmul(
                        ps[:, nh * 512:(nh + 1) * 512],
                        lhsT=wproj_sb[f][:, co * 128:(co + 1) * 128],
                        rhs=att_all[f][:, nh * 512:(nh + 1) * 512],
                        start=(f == 0), stop=(f == 1))
            ot = outp.tile([128, POS], F32, tag="out")
            nc.vector.tensor_scalar_add(out=ot[:], in0=ps[:], scalar1=bproj_sb[co][:])
            nc.sync.dma_start(out=out[FOUT_CONV + co * 128:FOUT_CONV + (co + 1) * 128, :],
                              in_=ot[:])

    nc.compile()
    _PROG_CACHE[("nc", variant)] = nc
    return nc


def _host_prep(x, w_kqv, b_kqv, w_proj, b_proj, w_conv, b_conv,
               key_rel_w, key_rel_h):
    """Layout-only host prep -> per-core input maps."""
    x = np.asarray(x, np.float32)
    w_kqv = np.asarray(w_kqv, np.float32)
    b_kqv = np.asarray(b_kqv, np.float32)
    w_proj = np.asarray(w_proj, np.float32)
    b_proj = np.asarray(b_proj, np.float32)
    w_conv = np.asarray(w_conv, np.float32)
    b_conv = np.asarray(b_conv, np.float32)
    key_rel_w = np.asarray(key_rel_w, np.float32)
    key_rel_h = np.asarray(key_rel_h, np.float32)

    scale = np.float32(DKH ** -0.5)
    wkqv = w_kqv.copy()
    wkqv[:, DK:2 * DK] *= scale           # fold q scaling into the weights
    bkq = b_kqv[:2 * DK].copy()
    bkq[DK:] *= scale
    # fold the v bias through the projection: attn = (attn0 + bv) Wp + bp
    bproj_eff = b_proj + b_kqv[2 * DK:] @ w_proj
    # combined per-partition bias tile [128, 8]:
    # cols 0-3 = b_kq 128-chunks, 4-5 = b_conv chunks, 6-7 = b_proj chunks
    ball = np.stack([bkq[0:128], bkq[128:256], bkq[256:384], bkq[384:512],
                     b_conv[0:128], b_conv[128:256],
                     bproj_eff[0:128], bproj_eff[128:256]], axis=1)

    # window-expanded relative tables, replicated to all 4 partition groups:
    #   krw4[32r + d, wq*32 + w'] = key_rel_w[w' - wq + 31, d]
    idx = (np.arange(W)[None, :] - np.arange(W)[:, None] + (W - 1))  # [wq, w']
    krw = key_rel_w[idx]                   # [wq, w', 32]
    krw4 = np.tile(krw.transpose(2, 0, 1).reshape(DKH, W * W), (4, 1))
    krh = key_rel_h[idx]
    krh4 = np.tile(krh.transpose(2, 0, 1).reshape(DKH, H * H), (4, 1))

    # one-hot offset deltas: rows 0-31 wk one-hots, rows 32-63 hk one-hots
    kpos = np.arange(POS)
    deltas = np.zeros((2 * W, POS), np.float32)
    deltas[kpos % W, kpos] = 1.0
    deltas[W + kpos // W, kpos] = 1.0

    # conv weights repacked so each 128-channel chunk's 9 taps are one
    # contiguous per-partition run: wconv[f][p, tp*256 + o]
    wc = w_conv.reshape(K * K, 2, 128, FOUT_CONV)          # [tap, f, p, o]
    wc = np.ascontiguousarray(wc.transpose(1, 2, 0, 3)).reshape(
        2, 128, K * K * FOUT_CONV)

    shared = {
        "wkqv": wkqv.astype(BF16),
        "wconv": wc.astype(BF16),
        "wproj": w_proj.astype(BF16),
        "biases": ball.astype(np.float32),
        "krw4": krw4.astype(BF16),
        "krh4": krh4.astype(BF16),
        "delta": deltas.astype(BF16),
    }
    PADW = H + 2
    in_maps = []
    for b in range(N_CORES):
        m = dict(shared)
        xt = np.ascontiguousarray(x[b].reshape(POS, FIN).T)   # [FIN, POS]
        xp = np.zeros((FIN, PADW, PADW), np.float32)
        xp[:, 1:H + 1, 1:W + 1] = xt.reshape(FIN, H, W)
        m["xpad"] = xp.reshape(FIN, PADW * PADW).astype(BF16)
        m["xtc"] = xt.astype(BF16)
        in_maps.append(m)
    return in_maps


def kernel(x, w_kqv, b_kqv, w_proj, b_proj, w_conv, b_conv,
           key_rel_w, key_rel_h):
    from concourse.bass_utils import run_bass_kernel_spmd

    nc = _build_program()
    in_maps = _host_prep(x, w_kqv, b_kqv, w_proj, b_proj, w_conv, b_conv,
                         key_rel_w, key_rel_h)
    res = run_bass_kernel_spmd(nc, in_maps, core_ids=list(range(N_CORES)))
    out = np.empty((B, H, W, FOUT), np.float32)
    for b in range(N_CORES):
        out[b] = res.results[b]["out"].T.reshape(H, W, FOUT)
    return out


# revision 42
# speedup vs baseline: 1.2590x; 1.2590x over previous
"""Attention-Augmented Conv2D (AAConv2D) distributed Bass kernel for 8 TRN2 NeuronCores.

Strategy: pure data-parallel over batch (B=8 -> one image per core, weights
replicated, zero collectives). Per core, for one [32,32,256] image:

  conv branch : 3x3 SAME conv (256->256ch) as 9 shifted-window matmuls
                accumulated in PSUM, channel-major output.
  attn branch : kqv 1x1 conv (channel-major k/q, position-major v),
                per-head S^T = K Q^T computed with an AUGMENTED contraction
                (K = 32 qk dims + 32 one-hot w-offset dims + 32 one-hot
                h-offset dims = 96) so the relative-position logits ride in
                the same matmul stream; exp on ScalarE (max-free softmax,
                logits are O(10) so fp32 exp is safe); P^T V via matmul with
                [v|1] stationary (sumexp ride-along); per-head normalize on
                VectorE; output projection.

All heavy matmuls in bf16 (fp32 matmul is 4x slower on TRN2 PE).
Host does layout-only prep: batch sharding, transposes to channel-major,
bf16 casts, relative-table window expansion, one-hot delta tables, and the
exact algebraic fold of the v-bias into the projection bias.
"""

import contextlib

import numpy as np
import ml_dtypes

BF16 = ml_dtypes.bfloat16

B, H, W, FIN = 8, 32, 32, 256
POS = H * W
FOUT, K, DK, DV, NH = 512, 3, 256, 256, 8
DKH, DVH = DK // NH, DV // NH
FOUT_CONV = FOUT - DV  # 256
N_CORES = 8

_PROG_CACHE = {}


def _build_program(variant="full"):
    """Build (and cache) the compiled Bass program. Same program for all 8
    cores (SPMD); per-core data arrives via the per-core input maps.

    variant: bisect knob —
      "noatt"  : skip the whole per-head attention loop
      "norel"  : attention without relative-logit matmuls
      "nonorm" : attention + rel, but plain copy instead of normalize
      "full"   : everything
    """
    if ("nc", variant) in _PROG_CACHE:
        return _PROG_CACHE[("nc", variant)]

    import concourse.bass as bass
    import concourse.bacc as bacc
    import concourse.tile as tile
    from concourse import mybir

    BF = mybir.dt.bfloat16
    F32 = mybir.dt.float32
    EXP = mybir.ActivationFunctionType.Exp

    nc = bacc.Bacc("TRN2", target_bir_lowering=False, debug=False,
                   num_devices=N_CORES)

    # ---- DRAM parameters (per-core shapes; all laid out for contiguous
    # per-partition DMA runs — descriptor count is a real startup cost) ----
    PADW = H + 2  # 34
    xpad_d = nc.dram_tensor("xpad", [FIN, PADW * PADW], BF, kind="ExternalInput")
    xtc_d = nc.dram_tensor("xtc", [FIN, POS], BF, kind="ExternalInput")
    wkqv_d = nc.dram_tensor("wkqv", [FIN, 2 * DK + DV], BF, kind="ExternalInput")
    wconv_d = nc.dram_tensor("wconv", [2, 128, K * K * FOUT_CONV], BF,
                             kind="ExternalInput")
    wproj_d = nc.dram_tensor("wproj", [DV, DV], BF, kind="ExternalInput")
    biases_d = nc.dram_tensor("biases", [128, 8], F32, kind="ExternalInput")
    krw_d = nc.dram_tensor("krw4", [128, 1024], BF, kind="ExternalInput")
    krh_d = nc.dram_tensor("krh4", [128, 1024], BF, kind="ExternalInput")
    delta_d = nc.dram_tensor("delta", [2 * W, POS], BF, kind="ExternalInput")
    out_d = nc.dram_tensor("out", [FOUT, POS], F32, kind="ExternalOutput")

    xpad_in = xpad_d.ap()
    xtc = xtc_d.ap()
    wkqv = wkqv_d.ap()
    wconv = wconv_d.ap()
    wproj = wproj_d.ap()
    biases = biases_d.ap()
    krw = krw_d.ap()
    krh = krh_d.ap()
    delta = delta_d.ap()
    out = out_d.ap()

    with tile.TileContext(nc) as tc, contextlib.ExitStack() as ctx:
        consts = ctx.enter_context(tc.tile_pool(name="consts", bufs=1))
        xpads = ctx.enter_context(tc.tile_pool(name="xpads", bufs=1))
        kqpool = ctx.enter_context(tc.tile_pool(name="kqpool", bufs=1))
        vopool = ctx.enter_context(tc.tile_pool(name="vopool", bufs=1))
        attall = ctx.enter_context(tc.tile_pool(name="attall", bufs=1))
        work = ctx.enter_context(tc.tile_pool(name="work", bufs=4))
        small = ctx.enter_context(tc.tile_pool(name="small", bufs=3))
        outp = ctx.enter_context(tc.tile_pool(name="outp", bufs=2))
        ps_big = ctx.enter_context(tc.tile_pool(name="ps_big", bufs=2,
                                                space=bass.MemorySpace.PSUM))
        ps_rp = ctx.enter_context(tc.tile_pool(name="ps_rp", bufs=1,
                                               space=bass.MemorySpace.PSUM))
        ps_at = ctx.enter_context(tc.tile_pool(name="ps_at", bufs=1,
                                               space=bass.MemorySpace.PSUM))

        # ---- zero-padded input (pre-padded on host), channel-major ----
        xpad = []
        xt_sb = []
        for f in range(2):
            t = xpads.tile([128, PADW * PADW], BF, tag=f"xpad{f}")
            nc.sync.dma_start(out=t[:], in_=xpad_in[f * 128:(f + 1) * 128, :])
            xpad.append(t)
            # unpadded copy: contiguous position axis (for matmul stationaries)
            tu = xpads.tile([128, POS], BF, tag=f"xtsb{f}")
            nc.sync.dma_start(out=tu[:], in_=xtc[f * 128:(f + 1) * 128, :])
            xt_sb.append(tu)

        # ---- constants into SBUF ----
        wkqv_sb = []
        for f in range(2):
            t = consts.tile([128, 2 * DK + DV], BF, tag=f"wkqv{f}")
            nc.sync.dma_start(out=t[:], in_=wkqv[f * 128:(f + 1) * 128, :])
            wkqv_sb.append(t)
        wconv_sb = []
        for f in range(2):
            t = consts.tile([128, K * K * FOUT_CONV], BF, tag=f"wconv{f}")
            nc.sync.dma_start(out=t[:], in_=wconv[f, :, :])
            wconv_sb.append(t)
        wproj_sb = []
        for f in range(2):
            t = consts.tile([128, DV], BF, tag=f"wproj{f}")
            nc.sync.dma_start(out=t[:], in_=wproj[f * 128:(f + 1) * 128, :])
            wproj_sb.append(t)
        krw_sb = consts.tile([128, 1024], BF, tag="krw")
        nc.sync.dma_start(out=krw_sb[:], in_=krw[:, :])
        krh_sb = consts.tile([128, 1024], BF, tag="krh")
        nc.sync.dma_start(out=krh_sb[:], in_=krh[:, :])

        # one combined per-partition bias tile: cols 0-3 = b_kq chunks,
        # 4-5 = b_conv chunks, 6-7 = effective b_proj chunks
        ball_sb = consts.tile([128, 8], F32, tag="ball")
        nc.sync.dma_start(out=ball_sb[:], in_=biases[:, :])
        bkq_sb = [ball_sb[:, cc:cc + 1] for cc in range(4)]
        bconv_sb = [ball_sb[:, 4 + co:5 + co] for co in range(2)]
        bproj_sb = [ball_sb[:, 6 + co:7 + co] for co in range(2)]

        def xwin(f, dy, dx, h0, hn):
            # [128, hn, 32] window of the padded image: rows h0..h0+hn of the
            # conv-tap (dy,dx)-shifted image, all 32 columns.
            t3 = xpad[f].rearrange("p (a b) -> p a b", a=PADW)
            return t3[:, h0 + dy:h0 + dy + hn, dx:dx + W]

        # ---- kqv: k and q sections, channel-major [co, pos] ----
        kq_sb = []
        for cc in range(4):
            ps = ps_big.tile([128, POS], F32, tag="psbig")
            for f in range(2):
                for nh in range(2):
                    nc.tensor.matmul(
                        ps[:, nh * 512:(nh + 1) * 512],
                        lhsT=wkqv_sb[f][:, cc * 128:(cc + 1) * 128],
                        rhs=xwin(f, 1, 1, nh * 16, 16),
                        start=(f == 0), stop=(f == 1))
            t = kqpool.tile([128, POS], BF, tag=f"kq{cc}")
            nc.vector.tensor_scalar_add(out=t[:], in0=ps[:], scalar1=bkq_sb[cc][:])
            kq_sb.append(t)

        # ---- v: position-major [pos, dv], with interleaved ones column ----
        vo_sb = []
        for kc in range(8):
            ps = ps_big.tile([128, DV], F32, tag="psbig")
            for f in range(2):
                nc.tensor.matmul(
                    ps[:],
                    lhsT=xt_sb[f][:, kc * 128:(kc + 1) * 128],
                    rhs=wkqv_sb[f][:, 2 * DK:2 * DK + DV],
                    start=(f == 0), stop=(f == 1))
            vo = vopool.tile([128, NH * (DVH + 1)], BF, tag=f"vo{kc}")
            vo3 = vo.rearrange("p (h d) -> p h d", d=DVH + 1)
            nc.vector.memset(vo3[:, :, DVH:DVH + 1], 1.0)
            nc.vector.tensor_copy(
                out=vo3[:, :, 0:DVH],
                in_=ps.rearrange("p (h d) -> p h d", d=DVH))
            vo_sb.append(vo)

        att_all = []
        for f in range(2):
            t = attall.tile([128, POS], BF, tag=f"att{f}", name=f"att{f}")
            att_all.append(t)

        # three persistent big-matmul stationaries (rows 32-95 = constant
        # one-hot deltas, loaded once; rows 0-31 swapped per head; three so
        # the prep-ahead depth of 2 never overwrites a stationary in use)
        st_trio = []
        for i in range(3):
            t = attall.tile([128, POS], BF, tag=f"stp{i}", name=f"stp{i}")
            nc.sync.dma_start(out=t[32:96, :], in_=delta[:, :])
            st_trio.append(t)
        ones_sb = consts.tile([1, 32], BF, tag="ones")
        nc.vector.memset(ones_sb[:], 1.0)

        # ---- per-head attention (software-pipelined: prep h+1 ahead) ----
        def prep_head(h):
            """rel matmuls + rhs/stationary assembly for head h."""
            sec = h // 4          # 128-channel chunk of the k/q section
            g = (h % 4) * 32      # partition offset of this head inside it

            # qT replicated to partition groups 0/1 (for row-tiled rel mms)
            qrep = work.tile([128, POS], BF, tag="qrep", name=f"qrep{h}")
            for r in range(2):
                nc.sync.dma_start(out=qrep[32 * r:32 * r + 32, :],
                                  in_=kq_sb[2 + sec][g:g + 32, :])

            # relative-position logit tables, gathered per (offset, q):
            #   rp[32+w', ...] = sum_d krw[d, wq*32+w'] qT[d, hq*32+wq]
            #   rp[64+h', ...] = sum_d krh[d, hq*32+h'] qT[d, hq*32+wq]
            # 2-way row-tiled packing; CONSTRAINT: concurrent tiles in the
            # same PE column strip must drain to DIFFERENT psum banks, so
            # the psum column is 512*(row group) + 32*(index//2).
            rp = ps_rp.tile([128, POS], F32, tag="rp", name=f"rp{h}")
            qrep3 = qrep.rearrange("p (a b) -> p b a", b=W)  # [128, wq, hq]
            for wq in range(W):
                r = wq % 2
                col = 512 * r + 32 * (wq // 2)
                nc.tensor.matmul(
                    rp[32:64, col:col + 32],
                    lhsT=krw_sb[32 * r:32 * r + 32, wq * 32:(wq + 1) * 32],
                    rhs=qrep3[32 * r:32 * r + 32, wq, :],
                    start=True, stop=True, tile_position=(32 * r, 32))
            for hq in range(H):
                r = hq % 2
                col = 512 * r + 32 * (hq // 2)
                nc.tensor.matmul(
                    rp[64:96, col:col + 32],
                    lhsT=krh_sb[32 * r:32 * r + 32, hq * 32:(hq + 1) * 32],
                    rhs=qrep[32 * r:32 * r + 32, hq * 32:(hq + 1) * 32],
                    start=True, stop=True, tile_position=(32 * r, 64))

            # rhs rows: 0-31 qT, 32-63 RWg, 64-95 RHg — un-permute into
            # q-major (one DVE copy per bank):
            #  rel_w: src col = 512b + 32a + hq  (wq = 2a + b); dst 32*hq + wq
            #  rel_h: src col = 512b + 32a + wq  (hq = 2a + b); dst 64a+32b+wq
            rh = work.tile([128, POS], BF, tag="rh", name=f"rh{h}")
            nc.sync.dma_start(out=rh[0:32, :], in_=kq_sb[2 + sec][g:g + 32, :])
            rw_src = rp[32:64, :].rearrange("p (b a c) -> p b c a", b=2, a=16)
            rw_dst = rh[32:64, :].rearrange("p (c a b) -> p b c a", a=16, b=2)
            rh_src = rp[64:96, :].rearrange("p (b a c) -> p b a c", b=2, a=16)
            rh_dst = rh[64:96, :].rearrange("p (a b c) -> p b a c", a=16, b=2)
            for bb in range(2):
                nc.vector.tensor_copy(out=rw_dst[:, bb], in_=rw_src[:, bb])
                nc.vector.tensor_copy(out=rh_dst[:, bb], in_=rh_src[:, bb])

            # stationary: swap in this head's kT rows (deltas persist)
            st = st_trio[h % 3]
            nc.sync.dma_start(out=st[0:32, :], in_=kq_sb[sec][g:g + 32, :])
            return rh, st

        def inner_head(h, rh, st):
            sec = h // 4
            g = (h % 4) * 32
            at = ps_at.tile([DVH + 1, POS], F32, tag="at", name=f"at{h}")
            # skewed S/PV emission: PE never sits behind the exp of the
            # chunk it just produced (S(kc+1) runs while ACT does exp(kc))
            sps_t = [None] * 8
            psb_t = [None] * 8

            def s_step(kc):
                sps = ps_big.tile([128, POS], F32, tag="psbig",
                                  name=f"sps{h}_{kc}")
                for nh in range(2):
                    nc.tensor.matmul(
                        sps[:, nh * 512:(nh + 1) * 512],
                        lhsT=st[0:96, kc * 128:(kc + 1) * 128],
                        rhs=rh[0:96, nh * 512:(nh + 1) * 512],
                        start=True, stop=True)
                psb = work.tile([128, POS], BF, tag="pexp", name=f"psb{h}_{kc}")
                nc.scalar.activation(out=psb[:], in_=sps[:], func=EXP)
                psb_t[kc] = psb

            def pv_step(kc):
                for nh in range(2):
                    nc.tensor.matmul(
                        at[:, nh * 512:(nh + 1) * 512],
                        lhsT=vo_sb[kc][:, h * (DVH + 1):(h + 1) * (DVH + 1)],
                        rhs=psb_t[kc][:, nh * 512:(nh + 1) * 512],
                        start=(kc == 0), stop=(kc == 7))

            s_step(0)
            for kc in range(1, 8):
                s_step(kc)
                pv_step(kc - 1)
            pv_step(7)

            # normalize: attn_h = (P^T V)[0:32] / sumexp (row 32).
            # One combined psum-escape copy frees `at` immediately; the
            # reciprocal runs on a [128, 8] reshape (DVE reciprocal cost is
            # free-dim-driven: [1,1024] would cost ~6.5us, [128,8] ~0.25us).
            if variant == "oldnorm":
                an = small.tile([32, POS], BF, tag="an", name=f"an{h}")
                rcp = small.tile([1, POS], F32, tag="rcp", name=f"rcp{h}")
                nc.vector.reciprocal(out=rcp[:], in_=at[DVH:DVH + 1, :])
                rcpb = small.tile([32, POS], F32, tag="rcpb", name=f"rcpb{h}")
                nc.gpsimd.partition_broadcast(rcpb[:], rcp[:])
                nc.vector.tensor_mul(an[:], at[0:DVH, :], rcpb[:])
                nc.sync.dma_start(out=att_all[sec][g:g + 32, :], in_=an[:])
                return
            cmb = small.tile([DVH + 1, POS], BF, tag="cmb", name=f"cmb{h}")
            nc.vector.tensor_copy(out=cmb[:], in_=at[:])
            s8 = small.tile([128, 8], BF, tag="s8", name=f"s8{h}")
            nc.gpsimd.dma_start(out=s8[:], in_=cmb[DVH:DVH + 1, :])
            rcp8 = small.tile([128, 8], BF, tag="rcp8", name=f"rcp8{h}")
            with nc.allow_low_precision(reason="1/sumexp in bf16 is within "
                                        "the softmax rounding budget"):
                nc.vector.reciprocal(out=rcp8[:], in_=s8[:])
            rcpf = small.tile([1, POS], BF, tag="rcpf", name=f"rcpf{h}")
            nc.gpsimd.dma_start(out=rcpf[:], in_=rcp8[:])
            # partition-broadcast 1/sumexp via a K=1 rank-1 matmul (the
            # GpSimd PartitionBroadcast op costs ~1.8us; this is ~0.5us on PE)
            rcpp = ps_rp.tile([32, POS], F32, tag="rp", name=f"rcpp{h}")
            for nh in range(2):
                nc.tensor.matmul(rcpp[:, nh * 512:(nh + 1) * 512],
                                 lhsT=ones_sb[:],
                                 rhs=rcpf[:, nh * 512:(nh + 1) * 512],
                                 start=True, stop=True)
            an = small.tile([32, POS], BF, tag="an", name=f"an{h}")
            nc.vector.tensor_mul(an[:], cmb[0:DVH, :], rcpp[:])
            nc.gpsimd.dma_start(out=att_all[sec][g:g + 32, :], in_=an[:])

        if variant != "noatt":
            # prep TWO heads ahead so the next head's PE work never waits on
            # the current head's normalize tail
            pq = [prep_head(0), prep_head(1)]
            for h in range(8):
                if h + 2 < 8:
                    pq.append(prep_head(h + 2))
                inner_head(h, *pq[h])
        else:
            for t in att_all:
                nc.vector.memset(t[:], 0.0)

        # ---- conv branch ----
        for co in range(2):
            ps = ps_big.tile([128, POS], F32, tag="psbig", name=f"cps{co}")
            for nh in range(2):
                idx = 0
                for tp in range(9):
                    dy, dx = tp // 3, tp % 3
                    for f in range(2):
                        o0 = tp * FOUT_CONV + co * 128
                        nc.tensor.matmul(
                            ps[:, nh * 512:(nh + 1) * 512],
                            lhsT=wconv_sb[f][:, o0:o0 + 128],
                            rhs=xwin(f, dy, dx, nh * 16, 16),
                            start=(idx == 0), stop=(idx == 17))
                        idx += 1
            ot = outp.tile([128, POS], F32, tag="out", name=f"cot{co}")
            nc.vector.tensor_scalar_add(out=ot[:], in0=ps[:], scalar1=bconv_sb[co][:])
            nc.sync.dma_start(out=out[co * 128:(co + 1) * 128, :], in_=ot[:])

        # ---- output projection ----
        for co in range(2):
            ps = ps_big.tile([128, POS], F32, tag="psbig")
            for f in range(2):
                for nh in range(2):
                    nc.tensor.matmul(
                        ps[:, nh * 512:(nh + 1) * 512],
                        lhsT=wproj_sb[f][:, co * 128:(co + 1) * 128],
                        rhs=att_all[f][:, nh * 512:(nh + 1) * 512],
                        start=(f == 0), stop=(f == 1))
            ot = outp.tile([128, POS], F32, tag="out")
            nc.vector.tensor_scalar_add(out=ot[:], in0=ps[:], scalar1=bproj_sb[co][:])
            nc.sync.dma_start(out=out[FOUT_CONV + co * 128:FOUT_CONV + (co + 1) * 128, :],
                              in_=ot[:])

    nc.compile()
    _PROG_CACHE[("nc", variant)] = nc
    return nc


def _host_prep(x, w_kqv, b_kqv, w_proj, b_proj, w_conv, b_conv,
               key_rel_w, key_rel_h):
    """Layout-only host prep -> per-core input maps."""
    x = np.asarray(x, np.float32)
    w_kqv = np.asarray(w_kqv, np.float32)
    b_kqv = np.asarray(b_kqv, np.float32)
    w_proj = np.asarray(w_proj, np.float32)
    b_proj = np.asarray(b_proj, np.float32)
    w_conv = np.asarray(w_conv, np.float32)
    b_conv = np.asarray(b_conv, np.float32)
    key_rel_w = np.asarray(key_rel_w, np.float32)
    key_rel_h = np.asarray(key_rel_h, np.float32)

    scale = np.float32(DKH ** -0.5)
    wkqv = w_kqv.copy()
    wkqv[:, DK:2 * DK] *= scale           # fold q scaling into the weights
    bkq = b_kqv[:2 * DK].copy()
    bkq[DK:] *= scale
    # fold the v bias through the projection: attn = (attn0 + bv) Wp + bp
    bproj_eff = b_proj + b_kqv[2 * DK:] @ w_proj
    # combined per-partition bias tile [128, 8]:
    # cols 0-3 = b_kq 128-chunks, 4-5 = b_conv chunks, 6-7 = b_proj chunks
    ball = np.stack([bkq[0:128], bkq[128:256], bkq[256:384], bkq[384:512],
                     b_conv[0:128], b_conv[128:256],
                     bproj_eff[0:128], bproj_eff[128:256]], axis=1)

    # window-expanded relative tables, replicated to all 4 partition groups:
    #   krw4[32r + d, wq*32 + w'] = key_rel_w[w' - wq + 31, d]
    idx = (np.arange(W)[None, :] - np.arange(W)[:, None] + (W - 1))  # [wq, w']
    krw = key_rel_w[idx]                   # [wq, w', 32]
    krw4 = np.tile(krw.transpose(2, 0, 1).reshape(DKH, W * W), (4, 1))
    krh = key_rel_h[idx]
    krh4 = np.tile(krh.transpose(2, 0, 1).reshape(DKH, H * H), (4, 1))

    # one-hot offset deltas: rows 0-31 wk one-hots, rows 32-63 hk one-hots
    kpos = np.arange(POS)
    deltas = np.zeros((2 * W, POS), np.float32)
    deltas[kpos % W, kpos] = 1.0
    deltas[W + kpos // W, kpos] = 1.0

    # conv weights repacked so each 128-channel chunk's 9 taps are one
    # contiguous per-partition run: wconv[f][p, tp*256 + o]
    wc = w_conv.reshape(K * K, 2, 128, FOUT_CONV)          # [tap, f, p, o]
    wc = np.ascontiguousarray(wc.transpose(1, 2, 0, 3)).reshape(
        2, 128, K * K * FOUT_CONV)

    shared = {
        "wkqv": wkqv.astype(BF16),
        "wconv": wc.astype(BF16),
        "wproj": w_proj.astype(BF16),
        "biases": ball.astype(np.float32),
        "krw4": krw4.astype(BF16),
        "krh4": krh4.astype(BF16),
        "delta": deltas.astype(BF16),
    }
    PADW = H + 2
    in_maps = []
    for b in range(N_CORES):
        m = dict(shared)
        xt = np.ascontiguousarray(x[b].reshape(POS, FIN).T)   # [FIN, POS]
        xp = np.zeros((FIN, PADW, PADW), np.float32)
        xp[:, 1:H + 1, 1:W + 1] = xt.reshape(FIN, H, W)
        m["xpad"] = xp.reshape(FIN, PADW * PADW).astype(BF16)
        m["xtc"] = xt.astype(BF16)
        in_maps.append(m)
    return in_maps


def kernel(x, w_kqv, b_kqv, w_proj, b_proj, w_conv, b_conv,
           key_rel_w, key_rel_h):
    from concourse.bass_utils import run_bass_kernel_spmd

    nc = _build_program()
    in_maps = _host_prep(x, w_kqv, b_kqv, w_proj, b_proj, w_conv, b_conv,
                         key_rel_w, key_rel_h)
    res = run_bass_kernel_spmd(nc, in_maps, core_ids=list(range(N_CORES)))
    out = np.empty((B, H, W, FOUT), np.float32)
    for b in range(N_CORES):
        out[b] = res.results[b]["out"].T.reshape(H, W, FOUT)
    return out
